# revision 1
# baseline (speedup 1.0000x reference)
"""Cluster-wise linear (MoE-style dense routing) Trainium2 kernel.

Computes out[t,o] = sum_c prob[t,c] * (x[t] @ W[c].T + b[c])[o] for
x (128,321,336) f32, prob (128,321,8), W (8,96,336), b (8,96).

Strategy: data-parallel over 8 NeuronCores (tokens = batch*n_vars split
evenly). Per core, 128-token tiles:
  - gpsimd DMA loads x with inline f32->bf16 cast
  - 3 TensorE transposes per tile put the contraction dim on partitions
    (the DMA-xbar transpose path measured ~3.2us/128x128, serialized --
    it was 90%% of kernel time; PE transposes hide behind the matmuls)
  - 6 bf16 matmuls accumulate Y[t, o*8+c] = (x|1) @ Wt_aug (bias folded
    in via a ones column; weights packed o-major on host)
  - ScalarE evicts Y PSUM->SBUF bf16
  - VectorE: Z = Y * prob (stride-0 broadcast), then strided reduce over
    the cluster axis -> out[t, o] in f32
"""

import numpy as np
import ml_dtypes

import concourse.bass as bass
import concourse.mybir as mybir
import concourse.tile as tile
from concourse.bass_utils import run_bass_kernel_spmd
from concourse.masks import make_identity

N_CORES = 8
BSZ, N_VARS, IN_DIM, OUT_DIM, N_CLUSTER = 128, 321, 336, 96, 8
TOK = BSZ * N_VARS            # 41088
TPC = TOK // N_CORES          # 5136 tokens per core
P = 128
N_TILES = (TPC + P - 1) // P  # 41 (40 full + 1 tail of 16)
TAIL = TPC - (N_TILES - 1) * P  # 16
IN_P = 384                    # padded input dim: 336 data + 1 ones + 47 zeros
CO = OUT_DIM * N_CLUSTER      # 768, o-major: co = o*8 + c


def split_multi_waits(nc):
    """This walrus build only supports one sync-wait per instruction; hoist
    extra waits onto same-engine nops inserted immediately before."""
    n_split = 0
    for fn in nc.m.functions:
        for bb in fn.blocks:
            insts = bb.instructions
            out = []
            changed = False
            for inst in insts:
                si = inst.sync_info
                if si is not None and si.on_wait and len(si.on_wait) > 1:
                    waits = list(si.on_wait)
                    del si.on_wait[1:]
                    si.on_wait[0] = waits[-1]
                    for w in waits[:-1]:
                        nop = mybir.InstNoOp(
                            name=f"{inst.name}-wsplit-{n_split}", ins=[], outs=[]
                        )
                        n_split += 1
                        nop.engine = inst.engine
                        nop.sync_info = mybir.SyncInfo(on_wait=[w], on_update=[])
                        out.append(nop)
                        changed = True
                out.append(inst)
            if changed:
                insts[:] = out
    return n_split


def build_nc(nrep: int = 1, bufs: int = 3, n_tiles: int = N_TILES, tail: int = TAIL, split_waits: bool = True,
             do_load=True, do_transpose=True, do_matmul=True, do_stage2=True,
             copyback_act=False):
    tpc = (n_tiles - 1) * P + tail
    nc = bass.Bass()
    x_d = nc.dram_tensor("x", [tpc, IN_DIM], mybir.dt.float32, kind="ExternalInput")
    p_d = nc.dram_tensor(
        "probp", [P, n_tiles * N_CLUSTER], mybir.dt.bfloat16, kind="ExternalInput"
    )
    w_d = nc.dram_tensor("wt", [IN_P, CO], mybir.dt.bfloat16, kind="ExternalInput")
    o_d = nc.dram_tensor("out", [tpc, OUT_DIM], mybir.dt.float32, kind="ExternalOutput")

    dt = mybir.dt
    with tile.TileContext(nc) as tc:
        with (
            tc.tile_pool(name="const", bufs=1) as const,
            tc.tile_pool(name="work", bufs=1) as work,
            tc.tile_pool(name="psum", bufs=1, space="PSUM") as psum,
        ):
            # one-time loads
            wtb = const.tile([P, 3 * CO], dt.bfloat16)
            wtb3 = wtb.rearrange("p (k n) -> p k n", k=3)
            nc.gpsimd.dma_start(wtb3[:], w_d.rearrange("(k p) n -> p k n", p=P))
            pball = const.tile([P, n_tiles * N_CLUSTER], dt.bfloat16)
            nc.gpsimd.dma_start(pball[:], p_d[:])
            pb3 = pball.rearrange("p (j c) -> p j c", c=N_CLUSTER)
            ident = const.tile([P, P], dt.bfloat16)
            make_identity(nc, ident[:])

            # rings
            xb_ring = [work.tile([P, IN_P], dt.bfloat16, name=f"xb{i}") for i in range(bufs)]
            xT_ring = [
                work.tile([P, 3 * P], dt.bfloat16, name=f"xT{i}") for i in range(bufs)
            ]
            tps_ring = [
                psum.tile([P, 3 * P], dt.bfloat16, name=f"tps{i}") for i in range(2)
            ]
            y_ring = [
                psum.tile([P, CO], dt.float32, name=f"yps{i}") for i in range(bufs)
            ]
            ysb_ring = [
                work.tile([P, CO], dt.bfloat16, name=f"ysb{i}") for i in range(bufs)
            ]
            z_ring = [work.tile([P, CO], dt.bfloat16, name=f"z{i}") for i in range(bufs)]
            o_ring = [
                work.tile([P, OUT_DIM], dt.float32, name=f"osb{i}") for i in range(bufs)
            ]
            # preset the ones column (bias row after transpose) and zero pad
            for xb in xb_ring:
                if do_load:
                    nc.vector.memset(xb[:, IN_DIM : IN_DIM + 1], 1.0)
                    nc.vector.memset(xb[:, IN_DIM + 1 :], 0.0)
                else:
                    nc.vector.memset(xb[:], 0.0)
            if not do_transpose:
                for t in xT_ring:
                    nc.vector.memset(t[:], 0.0)
            if not do_stage2:
                for t in o_ring:
                    nc.vector.memset(t[:], 0.0)

            def tile_body(j: int):
                h = P if j < n_tiles - 1 else tail
                t0 = j * P
                xb = xb_ring[j % bufs]
                if do_load:
                    nc.gpsimd.dma_start(xb[:h, 0:IN_DIM], x_d[t0 : t0 + h, :])
                xT = xT_ring[j % bufs]
                if do_transpose:
                    tps = tps_ring[j % 2]
                    for k in range(3):
                        nc.tensor.transpose(
                            tps[:, k * P : k * P + h],
                            xb[0:h, k * P : (k + 1) * P],
                            ident[0:h, 0:h],
                        )
                    if copyback_act:
                        nc.scalar.copy(xT[:], tps[:])
                    else:
                        nc.vector.tensor_copy(xT[:], tps[:])
                yps = y_ring[j % bufs]
                if do_matmul:
                    for k in range(3):
                        for n0, n1 in ((0, 512), (512, CO)):
                            nc.tensor.matmul(
                                yps[:h, n0:n1],
                                xT[:, k * P : k * P + h],
                                wtb3[:, k, n0:n1],
                                start=(k == 0),
                                stop=(k == 2),
                            )
                ysb = ysb_ring[j % bufs]
                osb = o_ring[j % bufs]
                if do_stage2:
                    nc.scalar.copy(ysb[:h, :], yps[:h, :])
                    z = z_ring[j % bufs]
                    zv = z[0:h].rearrange("p (o c) -> p o c", c=N_CLUSTER)
                    yv = ysb[0:h].rearrange("p (o c) -> p o c", c=N_CLUSTER)
                    pbc = pb3[0:h, j, :].unsqueeze(1).broadcast_to([h, OUT_DIM, N_CLUSTER])
                    nc.vector.tensor_tensor(zv, yv, pbc, mybir.AluOpType.mult)
                    nc.vector.tensor_reduce(
                        osb[0:h], zv, mybir.AxisListType.X, mybir.AluOpType.add
                    )
                nc.gpsimd.dma_start(o_d[t0 : t0 + h, :], osb[0:h])

            def sweep(_iv=None):
                for j in range(n_tiles):
                    tile_body(j)

            for _ in range(nrep):
                sweep()

    if split_waits:
        split_multi_waits(nc)
    return nc


def pack_inputs(x, prob, W, b):
    """Host-side packing. Returns per-core input maps."""
    x = np.asarray(x, dtype=np.float32).reshape(TOK, IN_DIM)
    prob = np.asarray(prob, dtype=np.float32).reshape(TOK, N_CLUSTER)
    W = np.asarray(W, dtype=np.float32)
    b = np.asarray(b, dtype=np.float32)

    # weights: wt[i, o*8+c] = W[c,o,i]; bias row at i=336; zeros to IN_P
    wt = np.zeros((IN_P, CO), dtype=np.float32)
    wt[:IN_DIM] = W.transpose(2, 1, 0).reshape(IN_DIM, CO)
    wt[IN_DIM] = b.T.reshape(CO)
    wt16 = np.ascontiguousarray(wt.astype(ml_dtypes.bfloat16))

    in_maps = []
    for c in range(N_CORES):
        xs = np.ascontiguousarray(x[c * TPC : (c + 1) * TPC])
        ps = prob[c * TPC : (c + 1) * TPC]
        pp = np.zeros((N_TILES * P, N_CLUSTER), dtype=np.float32)
        pp[:TPC] = ps
        # (j, p, c) -> (p, j, c)
        pp = pp.reshape(N_TILES, P, N_CLUSTER).transpose(1, 0, 2)
        pp16 = np.ascontiguousarray(
            pp.astype(ml_dtypes.bfloat16).reshape(P, N_TILES * N_CLUSTER)
        )
        in_maps.append({"x": xs, "probp": pp16, "wt": wt16})
    return in_maps


_cached = {}


def kernel(x, prob, W, b):
    key = "main"
    if key not in _cached:
        _cached[key] = build_nc(nrep=1)
    nc = _cached[key]
    in_maps = pack_inputs(x, prob, W, b)
    res = run_bass_kernel_spmd(nc, in_maps, list(range(N_CORES)))
    outs = [res.results[c]["out"] for c in range(N_CORES)]
    out = np.concatenate(outs, axis=0).reshape(BSZ, N_VARS, OUT_DIM)
    return out.astype(np.float32)


if __name__ == "__main__":
    rng = np.random.default_rng(0)
    x = rng.standard_normal((BSZ, N_VARS, IN_DIM)).astype(np.float32)
    prob = rng.random((BSZ, N_VARS, N_CLUSTER)).astype(np.float32)
    W = (rng.standard_normal((N_CLUSTER, OUT_DIM, IN_DIM)) / 18.3).astype(np.float32)
    b = rng.standard_normal((N_CLUSTER, OUT_DIM)).astype(np.float32) / 18.3
    out = kernel(x, prob, W, b)
    ref = np.einsum("ti,coi,tc->to", x.reshape(TOK, IN_DIM), W,
                    prob.reshape(TOK, N_CLUSTER)) + prob.reshape(TOK, N_CLUSTER) @ b
    ref = ref.reshape(BSZ, N_VARS, OUT_DIM)
    err = np.linalg.norm(out - ref) / np.linalg.norm(ref)
    print("rel_l2:", err)



# revision 12
# speedup vs baseline: 6.9952x; 6.9952x over previous
"""Cluster-wise linear (MoE-style dense routing) Trainium2 kernel, v2.

Computes out[t,o] = sum_c prob[t,c] * (x[t] @ W[c].T + b[c])[o] for
x (128,321,336) f32, prob (128,321,8), W (8,96,336), b (8,96).

Strategy: data-parallel over 8 NeuronCores (tokens = batch*n_vars split
evenly, 5136/core padded to 41 tiles of 128).

v2 changes vs v1:
  - x is transposed + cast to bf16 on the HOST (xT [384, 5248] with a
    ones row at 336 so the bias folds into the matmul). This removes the
    3 PE transposes + DVE copy per tile (384 of 2688 PE rows) and halves
    x HBM traffic. PE per tile = 6 matmuls = 2304 rows ~ 960ns.
  - x loads batched 4 tiles per DMA (1024B contiguous lines, avoids the
    <512B descriptor penalty), issued from sync (HWDGE). Output stores
    batched 2 tiles per DMA on gpsimd (SWDGE).
  - output written bf16 (host casts back to f32): halves out traffic and
    enables the DVE 2x mode on the cluster reduce.
Per-tile engine budget: PE 6 MM ~960ns (critical), Act copy ~780ns,
DVE mult+reduce ~660ns, DMA ~550ns.
"""

import numpy as np
import ml_dtypes

import concourse.bass as bass
import concourse.mybir as mybir
import concourse.tile as tile
from concourse.bass_utils import run_bass_kernel_spmd

N_CORES = 8
BSZ, N_VARS, IN_DIM, OUT_DIM, N_CLUSTER = 128, 321, 336, 96, 8
TOK = BSZ * N_VARS            # 41088
TPC = TOK // N_CORES          # 5136 tokens per core
P = 128
N_TILES = (TPC + P - 1) // P  # 41
TPAD = N_TILES * P            # 5248 padded tokens per core
N_PAIR = (N_TILES + 1) // 2   # 21 output pairs
OPAD = N_PAIR * 2 * P         # 5376 padded rows in the out tensor
IN_P = 384                    # padded input dim: 336 data + 1 ones + 47 zeros
CO = OUT_DIM * N_CLUSTER      # 768, o-major: co = o*8 + c
XBLK = 4                      # tiles per x-load DMA
N_XBLK = N_TILES // XBLK      # 10 full blocks
XREM = N_TILES - N_XBLK * XBLK  # 1 leftover tile


def split_multi_waits(nc):
    """This walrus build only supports one sync-wait per instruction; hoist
    extra waits onto same-engine nops inserted immediately before."""
    n_split = 0
    for fn in nc.m.functions:
        for bb in fn.blocks:
            insts = bb.instructions
            out = []
            changed = False
            for inst in insts:
                si = inst.sync_info
                if si is not None and si.on_wait and len(si.on_wait) > 1:
                    waits = list(si.on_wait)
                    del si.on_wait[1:]
                    si.on_wait[0] = waits[-1]
                    for w in waits[:-1]:
                        nop = mybir.InstNoOp(
                            name=f"{inst.name}-wsplit-{n_split}", ins=[], outs=[]
                        )
                        n_split += 1
                        nop.engine = inst.engine
                        nop.sync_info = mybir.SyncInfo(on_wait=[w], on_update=[])
                        out.append(nop)
                        changed = True
                out.append(inst)
            if changed:
                insts[:] = out
    return n_split


def build_nc(nrep: int = 1, bufs: int = 3, loop_iters: int | None = None,
             do_load=True, do_matmul=True, do_stage2=True, do_store=True,
             x_dma_sync=True, out_dma_sync=False):
    nc = bass.Bass()
    dt = mybir.dt
    x_d = nc.dram_tensor("xt", [IN_P, TPAD], dt.bfloat16, kind="ExternalInput")
    p_d = nc.dram_tensor(
        "probp", [P, N_TILES * N_CLUSTER], dt.bfloat16, kind="ExternalInput"
    )
    w_d = nc.dram_tensor("wt", [IN_P, CO], dt.bfloat16, kind="ExternalInput")
    o_d = nc.dram_tensor("out", [OPAD, OUT_DIM], dt.bfloat16, kind="ExternalOutput")

    x_eng = nc.sync if x_dma_sync else nc.gpsimd

    with tile.TileContext(nc) as tc:
        with (
            tc.tile_pool(name="const", bufs=1) as const,
            tc.tile_pool(name="work", bufs=1) as work,
            tc.tile_pool(name="psum", bufs=1, space="PSUM") as psum,
        ):
            # one-time loads
            wtb = const.tile([P, 3 * CO], dt.bfloat16)
            wtb3 = wtb.rearrange("p (k n) -> p k n", k=3)
            nc.gpsimd.dma_start(wtb3[:], w_d.rearrange("(k p) n -> p k n", p=P))
            pball = const.tile([P, N_TILES * N_CLUSTER], dt.bfloat16)
            nc.gpsimd.dma_start(pball[:], p_d[:])
            pb3 = pball.rearrange("p (j c) -> p j c", c=N_CLUSTER)

            # rings
            xb_ring = [
                work.tile([P, 3 * XBLK * P], dt.bfloat16, name=f"xb{i}")
                for i in range(2)
            ]
            xb_v = [t.rearrange("p (k t) -> p k t", k=3) for t in xb_ring]
            y_ring = [
                psum.tile([P, CO], dt.float32, name=f"yps{i}") for i in range(bufs)
            ]
            ysb_ring = [
                work.tile([P, CO], dt.bfloat16, name=f"ysb{i}") for i in range(bufs)
            ]
            z_ring = [work.tile([P, CO], dt.bfloat16, name=f"z{i}") for i in range(bufs)]
            o_ring = [
                work.tile([P, 2 * OUT_DIM], dt.bfloat16, name=f"osb{i}")
                for i in range(2)
            ]
            o_v = [t.rearrange("p (h o) -> p h o", h=2) for t in o_ring]

            # DRAM views
            x_full = x_d.rearrange("(k p) t -> p k t", p=P)
            o_blk = o_d.rearrange("(q h p) o -> p q h o", h=2, p=P)

            # the lone final tile stores a half-garbage pair into padded
            # DRAM rows; memset once so the padding lane is finite
            for t in o_ring:
                nc.vector.memset(t[:], 0.0)

            def tile_body(j: int):
                if do_load:
                    if j % XBLK == 0 and j // XBLK < N_XBLK:
                        q = j // XBLK
                        xb = xb_v[q % 2]
                        x_eng.dma_start(
                            xb[:], x_full[:, :, q * XBLK * P : (q + 1) * XBLK * P]
                        )
                    elif j == N_XBLK * XBLK:
                        # leftover tile: load into first part of next ring slot
                        xb = xb_v[(j // XBLK) % 2]
                        x_eng.dma_start(
                            xb[:, :, 0:P], x_full[:, :, j * P : (j + 1) * P]
                        )
                xb = xb_v[(j // XBLK) % 2]
                m0 = (j % XBLK) * P if j < N_XBLK * XBLK else 0
                yps = y_ring[j % bufs]
                if do_matmul:
                    for k in range(3):
                        for n0, n1 in ((0, 512), (512, CO)):
                            nc.tensor.matmul(
                                yps[:, n0:n1],
                                xb[:, k, m0 : m0 + P],
                                wtb3[:, k, n0:n1],
                                start=(k == 0),
                                stop=(k == 2),
                            )
                if do_stage2:
                    ysb = ysb_ring[j % bufs]
                    nc.scalar.copy(ysb[:], yps[:])
                    z = z_ring[j % bufs]
                    zv = z.rearrange("p (o c) -> p o c", c=N_CLUSTER)
                    yv = ysb.rearrange("p (o c) -> p o c", c=N_CLUSTER)
                    pbc = pb3[:, j, :].unsqueeze(1).broadcast_to([P, OUT_DIM, N_CLUSTER])
                    nc.vector.tensor_tensor(zv, yv, pbc, mybir.AluOpType.mult)
                    with nc.allow_low_precision(
                        reason="8-term cluster sum in bf16; error ~0.3% vs 2e-2 gate"
                    ):
                        nc.vector.tensor_reduce(
                            o_v[(j // 2) % 2][:, j % 2, :], zv,
                            mybir.AxisListType.X, mybir.AluOpType.add,
                        )
                if do_store and (j % 2 == 1 or j == N_TILES - 1):
                    nc.gpsimd.dma_start(
                        o_blk[:, j // 2, :, :], o_v[(j // 2) % 2][:]
                    )

            if loop_iters is not None:
                # hardware loop for fast dev timing (per-iter all-engine
                # barrier adds a constant bias; use for relative comparisons)
                with tc.For_i(0, loop_iters):
                    for j in range(N_TILES):
                        tile_body(j)
            else:
                for _ in range(nrep):
                    for j in range(N_TILES):
                        tile_body(j)

    split_multi_waits(nc)
    return nc


def pack_inputs(x, prob, W, b):
    """Host-side packing. Returns per-core input maps."""
    x = np.asarray(x, dtype=np.float32).reshape(TOK, IN_DIM)
    prob = np.asarray(prob, dtype=np.float32).reshape(TOK, N_CLUSTER)
    W = np.asarray(W, dtype=np.float32)
    b = np.asarray(b, dtype=np.float32)

    # weights: wt[i, o*8+c] = W[c,o,i]; bias row at i=336; zeros to IN_P
    wt = np.zeros((IN_P, CO), dtype=np.float32)
    wt[:IN_DIM] = W.transpose(2, 1, 0).reshape(IN_DIM, CO)
    wt[IN_DIM] = b.T.reshape(CO)
    wt16 = np.ascontiguousarray(wt.astype(ml_dtypes.bfloat16))

    in_maps = []
    for c in range(N_CORES):
        xs = x[c * TPC : (c + 1) * TPC]
        # xT [384, 5248] bf16: rows 0:336 = x.T, row 336 = ones, rest zero
        xT = np.zeros((IN_P, TPAD), dtype=ml_dtypes.bfloat16)
        xT[:IN_DIM, :TPC] = xs.T.astype(ml_dtypes.bfloat16)
        xT[IN_DIM, :] = ml_dtypes.bfloat16(1.0)
        xT = np.ascontiguousarray(xT)

        ps = prob[c * TPC : (c + 1) * TPC]
        pp = np.zeros((TPAD, N_CLUSTER), dtype=np.float32)
        pp[:TPC] = ps
        # (j, p, c) -> (p, j, c)
        pp = pp.reshape(N_TILES, P, N_CLUSTER).transpose(1, 0, 2)
        pp16 = np.ascontiguousarray(
            pp.astype(ml_dtypes.bfloat16).reshape(P, N_TILES * N_CLUSTER)
        )
        in_maps.append({"xt": xT, "probp": pp16, "wt": wt16})
    return in_maps


_cached = {}


def kernel(x, prob, W, b):
    key = "main"
    if key not in _cached:
        _cached[key] = build_nc(nrep=1)
    nc = _cached[key]
    in_maps = pack_inputs(x, prob, W, b)
    res = run_bass_kernel_spmd(nc, in_maps, list(range(N_CORES)))
    outs = [res.results[c]["out"][:TPC] for c in range(N_CORES)]
    out = np.concatenate(outs, axis=0).astype(np.float32).reshape(
        BSZ, N_VARS, OUT_DIM
    )
    return out


if __name__ == "__main__":
    rng = np.random.default_rng(0)
    x = rng.standard_normal((BSZ, N_VARS, IN_DIM)).astype(np.float32)
    prob = rng.random((BSZ, N_VARS, N_CLUSTER)).astype(np.float32)
    W = (rng.standard_normal((N_CLUSTER, OUT_DIM, IN_DIM)) / 18.3).astype(np.float32)
    b = rng.standard_normal((N_CLUSTER, OUT_DIM)).astype(np.float32) / 18.3
    out = kernel(x, prob, W, b)
    ref = np.einsum("ti,coi,tc->to", x.reshape(TOK, IN_DIM), W,
                    prob.reshape(TOK, N_CLUSTER)) + prob.reshape(TOK, N_CLUSTER) @ b
    ref = ref.reshape(BSZ, N_VARS, OUT_DIM)
    err = np.linalg.norm(out - ref) / np.linalg.norm(ref)
    print("rel_l2:", err)
